# revision 1
# baseline (speedup 1.0000x reference)
"""Multi-head attention (B=2, N=2048, d_model=1024, H=16) on 8 NeuronCores.

Sharding: data-parallel on batch (2) x tensor-parallel on heads (4 groups of
4 heads). Core c handles batch c//4, head-group c%4. Each core computes its
heads' Q/K/V projections, causal attention, and a partial output projection;
the host sums the 4 partials per batch.

All matmuls run in bf16 with fp32 PSUM accumulation. Softmax skips the
max-subtraction (scores here are bounded by ~+-5, exp is safe) so attention
needs no transposes: scores are computed directly in S.T orientation
[keys, queries], exp'd, and fed to PV as the stationary operand with a
ones-column on V producing the softmax denominator for free.

Engine budget per core: PE ~117us of matmul, ACT ~66us of exp (the softmax
exp at 1 elem/cycle/lane is the secondary bottleneck, so ACT does nothing
else on the attention path), DVE does all PSUM->SBUF copies and the
normalization, GPSIMD broadcasts the denominators.
"""

import sys

if "/opt/trn_rl_repo" not in sys.path:
    sys.path.insert(0, "/opt/trn_rl_repo")

import numpy as np
import ml_dtypes

import concourse.bass as bass
import concourse.mybir as mybir
import concourse.tile as tile
from concourse import bacc
from concourse.bass_utils import run_bass_kernel_spmd
from concourse.masks import make_upper_triangular

B, N, D, H = 2, 2048, 1024, 16
DV = D // H  # 64
HPC = H // 4  # heads per core: 4
DHC = HPC * DV  # head dims per core: 256
NT = N // 128  # 16 m-tiles
NC = N // 512  # 4 n-chunks
DT = D // 128  # 8 din-tiles
BF = mybir.dt.bfloat16
F32 = mybir.dt.float32
EXP = mybir.ActivationFunctionType.Exp
SCALE = 0.125  # 1/sqrt(DV)

_CACHE = {}


def build_nc():
    nc = bacc.Bacc("TRN2", target_bir_lowering=False, debug=False)
    xqT_d = nc.dram_tensor("xqT", [D, N], BF, kind="ExternalInput")
    xkT_d = nc.dram_tensor("xkT", [D, N], BF, kind="ExternalInput")
    xvT_d = nc.dram_tensor("xvT", [D, N], BF, kind="ExternalInput")
    wqT_d = nc.dram_tensor("wqT", [D, DHC], BF, kind="ExternalInput")
    woT_d = nc.dram_tensor("woT", [DHC, D], BF, kind="ExternalInput")
    bq_d = nc.dram_tensor("bq", [DHC], F32, kind="ExternalInput")
    yT_d = nc.dram_tensor("yT", [D, N], mybir.dt.float16, kind="ExternalOutput")

    with tile.TileContext(nc) as tc:
        with (
            tc.tile_pool(name="consts", bufs=1) as consts,
            tc.tile_pool(name="xin", bufs=1) as xin,
            tc.tile_pool(name="prod", bufs=1) as prod,
            tc.tile_pool(name="work", bufs=3) as work,
            tc.tile_pool(name="norm", bufs=3) as norm,
            tc.tile_pool(name="yout", bufs=2) as yout,
            tc.tile_pool(name="ps", bufs=1, space="PSUM") as ps,
        ):
            # ---- weights + constants (small, load first) ----
            wqT = consts.tile([128, DT, DHC], BF, name="wqT")
            nc.sync.dma_start(
                out=wqT, in_=wqT_d.ap().rearrange("(j p) c -> p j c", p=128)
            )
            bq_pp = consts.tile([128, 2], F32, name="bq_pp")
            nc.sync.dma_start(
                out=bq_pp, in_=bq_d.ap().rearrange("(c p) -> p c", p=128)
            )
            bq_row = consts.tile([1, DHC], F32, name="bq_row")
            nc.sync.dma_start(
                out=bq_row, in_=bq_d.ap().rearrange("(a c) -> a c", a=1)
            )
            bq_bc = consts.tile([128, DHC], F32, name="bq_bc")
            nc.gpsimd.partition_broadcast(bq_bc, bq_row)
            utmask = consts.tile([128, 128], BF, name="utmask")
            make_upper_triangular(nc, utmask, val=1.0, diag=True)

            # ---- bulk inputs: per-j-tile DMAs so projection matmuls can
            # start as tiles land; k first (kT-proj fills the load window) ----
            xkT = xin.tile([128, DT, N], BF, name="xkT")
            xqT = xin.tile([128, DT, N], BF, name="xqT")
            xvT = xin.tile([128, DT, N], BF, name="xvT")
            # n-sliced loads ordered by first use: xq/xk chunk-0 gate the
            # first S matmul, xv chunk-0 gates the first PV
            def load_slice(t, d, n0, n1):
                nc.sync.dma_start(
                    out=t[:, :, n0:n1],
                    in_=d.ap()[:, n0:n1].rearrange("(j p) n -> p j n", p=128),
                )

            def load_j2(t, d, j2):
                nc.sync.dma_start(
                    out=t[:, j2 : j2 + 2, :],
                    in_=d.ap()[j2 * 128 : (j2 + 2) * 128, :].rearrange(
                        "(j p) n -> p j n", p=128
                    ),
                )

            for j2 in range(0, DT, 2):
                load_j2(xkT, xkT_d, j2)
            load_slice(xqT, xqT_d, 0, 512)
            load_slice(xvT, xvT_d, 0, 512)
            load_slice(xqT, xqT_d, 512, N)
            for s in range(1, 4):
                load_slice(xvT, xvT_d, s * 512, (s + 1) * 512)
            woT = consts.tile([128, 2, D], BF, name="woT")
            nc.sync.dma_start(
                out=woT, in_=woT_d.ap().rearrange("(q p) c -> p q c", p=128)
            )

            # ---- unit-pipelined schedule ----
            # Attention "units" are (chunk, head). Per unit: burst of S
            # matmuls + exps, with projection/outproj groups sprinkled in as
            # PE filler, then the PV burst for the *previous* unit (whose
            # exps are long done -> no PE-waiting-on-ACT bubbles).
            vp = [
                prod.tile([128, HPC, DV + 1], BF, name=f"vp{m}")
                for m in range(NT)
            ]
            qT = [prod.tile([128, N], BF, name=f"qT{p}") for p in range(2)]
            kT = [prod.tile([128, N], BF, name=f"kT{p}") for p in range(2)]
            xaT = [prod.tile([128, N], BF, name=f"xaT{p}") for p in range(2)]

            def proj_qk(src_t, dst, c, p):
                pp = ps.tile([128, 512], F32, name="prj_qk", tag="prj", bufs=2)
                for j in range(DT):
                    nc.tensor.matmul(
                        pp,
                        wqT[:, j, p * 128 : (p + 1) * 128],
                        src_t[:, j, c * 512 : (c + 1) * 512],
                        start=(j == 0),
                        stop=(j == DT - 1),
                    )
                nc.vector.tensor_scalar_add(
                    dst[p][:, c * 512 : (c + 1) * 512], pp, bq_pp[:, p : p + 1]
                )

            def proj_v(m):
                pv = ps.tile([128, 512], F32, name="prj_v", tag="prj", bufs=2)
                pvv = pv[:, 0:DHC]
                for j in range(DT):
                    nc.tensor.matmul(
                        pvv,
                        xvT[:, j, m * 128 : (m + 1) * 128],
                        wqT[:, j, :],
                        start=(j == 0),
                        stop=(j == DT - 1),
                    )
                nc.vector.tensor_add(
                    vp[m][:, :, 0:DV],
                    pvv.rearrange("p (h d) -> p h d", h=HPC),
                    bq_bc.rearrange("p (h d) -> p h d", h=HPC),
                )
                nc.vector.memset(vp[m][:, :, DV : DV + 1], 1.0)

            def outproj_t(c, t, act_copy=False):
                yp = ps.tile([128, 512], F32, name="yp", tag="prj", bufs=2)
                for p in range(2):
                    nc.tensor.matmul(
                        yp,
                        woT[:, p, t * 128 : (t + 1) * 128],
                        xaT[p][:, c * 512 : (c + 1) * 512],
                        start=(p == 0),
                        stop=(p == 1),
                    )
                y_sb = yout.tile(
                    [128, 512], mybir.dt.float16, name=f"y_sb{t}",
                    tag=f"y{t % 4}",
                )
                if act_copy:
                    nc.scalar.copy(y_sb, yp)
                else:
                    nc.vector.tensor_copy(y_sb, yp)
                nc.sync.dma_start(
                    out=yT_d.ap()[
                        t * 128 : (t + 1) * 128, c * 512 : (c + 1) * 512
                    ],
                    in_=y_sb,
                )

            # pT tiles for unit u are consumed by PV in the next unit
            pT_tiles = {}

            def s_exp_burst(c, hp, fillers):
                """S+exp burst for a HEAD PAIR (heads 2hp, 2hp+1).

                The two heads' S matmuls contract over disjoint PE row groups
                (array rows 0-63 vs 64-127, from the operands' base
                partitions), so emitting them back-to-back lets the PE run
                them concurrently. Both land in one [128,1024] psum and share
                a single wide exp."""
                jmax = 4 * c + 3
                fi = list(fillers)
                for j in range(jmax + 1):
                    off = max(0, (j - 4 * c) * 128)
                    w = 512 - off
                    sp = ps.tile([128, 1024], F32, name="sp", tag="sp", bufs=2)
                    pT = work.tile(
                        [128, 1024], BF, name="pT", tag="pT", bufs=22
                    )
                    for hr in range(2):
                        nc.tensor.matmul(
                            sp[:, hr * 512 : hr * 512 + w],
                            kT[hp][
                                hr * 64 : (hr + 1) * 64,
                                j * 128 : (j + 1) * 128,
                            ],
                            qT[hp][
                                hr * 64 : (hr + 1) * 64,
                                c * 512 + off : (c + 1) * 512,
                            ],
                            start=True,
                            stop=True,
                            skip_group_check=True,
                        )
                    if off:
                        # diag block: exp only the two valid [0,w) regions
                        spv = sp.rearrange("p (b k) -> p b k", b=2)[:, :, 0:w]
                        pTv = pT.rearrange("p (b k) -> p b k", b=2)[:, :, 0:w]
                        nc.scalar.activation(pTv, spv, EXP, scale=SCALE)
                    else:
                        nc.scalar.activation(pT, sp, EXP, scale=SCALE)
                    for hr in range(2):
                        if j >= 4 * c:  # diagonal block: causal mask
                            nc.vector.tensor_mul(
                                pT[:, hr * 512 : hr * 512 + 128],
                                pT[:, hr * 512 : hr * 512 + 128],
                                utmask,
                            )
                        pT_tiles[(c, 2 * hp + hr, j)] = pT[
                            :, hr * 512 : hr * 512 + 512
                        ]
                    if j % 2 and fi:
                        fi.pop(0)()
                for f in fi:
                    f()

            def pv_norm_pair(c, hp):
                jmax = 4 * c + 3
                for hr in range(2):
                    h = 2 * hp + hr
                    op = ps.tile([DV + 1, 512], F32, name="op", tag="op", bufs=2)
                    for j in range(jmax + 1):
                        off = max(0, (j - 4 * c) * 128)
                        w = 512 - off
                        pT = pT_tiles.pop((c, h, j))
                        nc.tensor.matmul(
                            op[:, off:512],
                            vp[j][:, h, :],
                            pT[:, 0:w],
                            start=(j == 0),
                            stop=(j == jmax),
                        )
                    # normalize: rows 0:64 /= row 64 (softmax denominator)
                    rrow = norm.tile([1, 512], F32, name="rrow", tag="rrow")
                    nc.vector.reciprocal(rrow, op[DV : DV + 1, :])
                    rrec = norm.tile([64, 512], F32, name="rrec", tag="rrec")
                    nc.gpsimd.partition_broadcast(rrec, rrow)
                    nc.vector.tensor_mul(
                        xaT[hp][hr * 64 : (hr + 1) * 64, c * 512 : (c + 1) * 512],
                        op[0:DV, :],
                        rrec,
                    )

            def F(fn, *a):
                return lambda: fn(*a)

            fillers = {
                (0, 0): [
                    F(proj_qk, xkT, kT, 1, 0),
                    F(proj_qk, xkT, kT, 1, 1),
                    F(proj_qk, xkT, kT, 3, 0),
                    F(proj_qk, xkT, kT, 3, 1),
                ],
                (0, 1): [
                    F(proj_qk, xkT, kT, 2, 0),
                    F(proj_qk, xkT, kT, 2, 1),
                    F(proj_qk, xqT, qT, 3, 0),
                    F(proj_qk, xqT, qT, 3, 1),
                ],
                (3, 0): [F(proj_v, m) for m in range(4, 16)],
                (3, 1): [F(outproj_t, 0, t) for t in range(4)]
                + [F(proj_qk, xqT, qT, 2, 0), F(proj_qk, xqT, qT, 2, 1)],
                (2, 0): [F(outproj_t, 0, t) for t in range(4, 8)],
                (2, 1): [F(outproj_t, 3, t) for t in range(4)]
                + [F(proj_qk, xqT, qT, 1, 0), F(proj_qk, xqT, qT, 1, 1)],
                (1, 0): [F(outproj_t, 3, t) for t in range(4, 8)],
                (1, 1): [F(outproj_t, 2, t) for t in range(6)],
            }

            # prologue: chunk-0 projections; later kT/qT chunks are fillers
            for p in range(2):
                proj_qk(xkT, kT, 0, p)
            for p in range(2):
                proj_qk(xqT, qT, 0, p)
            s_exp_burst(0, 0, fillers[(0, 0)])
            for m in range(4):
                proj_v(m)

            units = [(0, 1), (3, 0), (3, 1), (2, 0), (2, 1), (1, 0), (1, 1)]
            prev = (0, 0)
            for cu in units:
                s_exp_burst(*cu, fillers[cu])
                pv_norm_pair(*prev)
                prev = cu
            pv_norm_pair(*prev)
            for t in (6, 7):
                outproj_t(2, t)
            for t in range(DT):
                outproj_t(1, t, act_copy=bool(t % 2))
    nc.compile()
    return nc


def kernel(**inputs):
    inputs = {k: np.asarray(v) for k, v in inputs.items()}
    Q, K, V = inputs["Q"], inputs["K"], inputs["V"]
    wq, bq, wo, bo = inputs["wq"], inputs["bq"], inputs["wo"], inputs["bo"]

    def bfT(x):  # bf16 transpose [n, d] -> [d, n]
        return np.ascontiguousarray(x.astype(ml_dtypes.bfloat16).T)

    xqT = [bfT(Q[b]) for b in range(B)]
    xkT = [bfT(K[b]) for b in range(B)]
    xvT = [bfT(V[b]) for b in range(B)]
    wqT = [bfT(wq[g * DHC : (g + 1) * DHC, :]) for g in range(4)]
    woT = [bfT(wo[:, g * DHC : (g + 1) * DHC]) for g in range(4)]
    bqs = [np.ascontiguousarray(bq[g * DHC : (g + 1) * DHC], dtype=np.float32)
           for g in range(4)]

    if "nc" not in _CACHE:
        _CACHE["nc"] = build_nc()
    nc = _CACHE["nc"]

    in_maps = []
    for core in range(8):
        b, g = divmod(core, 4)
        in_maps.append(
            {
                "xqT": xqT[b],
                "xkT": xkT[b],
                "xvT": xvT[b],
                "wqT": wqT[g],
                "woT": woT[g],
                "bq": bqs[g],
            }
        )
    import os

    trace = bool(int(os.environ.get("KERNEL_TRACE", "0")))
    try:
        res = run_bass_kernel_spmd(
            nc, in_maps, core_ids=list(range(8)), trace=trace
        )
    except ModuleNotFoundError:
        # NTFF profiling hook unavailable in this environment
        res = run_bass_kernel_spmd(nc, in_maps, core_ids=list(range(8)))
    _CACHE["last_results"] = res

    out = np.empty((B, N, D), np.float32)
    for b in range(B):
        acc = res.results[4 * b]["yT"].astype(np.float32)
        for g in range(1, 4):
            acc += res.results[4 * b + g]["yT"]
        out[b] = acc.T + bo
    return out



# revision 11
# speedup vs baseline: 1.1663x; 1.1663x over previous
"""Multi-head attention (B=2, N=2048, d_model=1024, H=16) on 8 NeuronCores.

Sharding: data-parallel on batch (2) x tensor-parallel on heads (4 groups of
4 heads). Core c handles batch c//4, head-group c%4. Each core computes its
heads' Q/K/V projections, causal attention, and a partial output projection;
the host sums the 4 partials per batch.

Key structure (v2):
- Q/K/V projections run in fp8e4 DoubleRow perf mode with hi/lo error
  compensation: x ~ x_hi + x_lo, w ~ w_hi + w_lo (all e4m3, split on host),
  and the three products hh+hl+lh accumulate in one PSUM group. 2 k-tiles
  pack per DoubleRow matmul at 0.5 cycles/row -> ~2.7x faster than bf16 at
  bf16-like accuracy (lo*lo term ~0.1%).
- Scores are computed in S.T orientation [keys, queries] per head-pair
  (disjoint PE row groups), exp'd on ACT without max-subtraction (scores
  bounded), causal-masked by an upper-tri multiply on the diagonal blocks.
- PV is "flipped": P blocks [128k x 128q] are the stationary operand and
  the (bf16) V tile [128k x 65] streams, so each key-block step costs 65
  rows instead of 512. Column 64 of V is ones -> softmax denominator rides
  along. Output lands as [q, dh] + per-partition scalar normalize on DVE,
  then a PE transpose puts it back in [dh, q] for the output projection.
- Output projection in bf16; PSUM->SBUF f16 copies split across DVE /
  GPSIMD / ACT; partial sums of the 4 head-groups folded on host.
"""

import sys

if "/opt/trn_rl_repo" not in sys.path:
    sys.path.insert(0, "/opt/trn_rl_repo")

import numpy as np
import ml_dtypes

import concourse.bass as bass
import concourse.mybir as mybir
import concourse.tile as tile
from concourse import bacc
from concourse.bass_utils import run_bass_kernel_spmd
from concourse.masks import make_upper_triangular, make_identity

B, N, D, H = 2, 2048, 1024, 16
DV = D // H  # 64
HPC = H // 4  # heads per core: 4
DHC = HPC * DV  # head dims per core: 256
NT = N // 128  # 16 m-tiles
DT = D // 128  # 8 din-tiles
BF = mybir.dt.bfloat16
F8 = mybir.dt.float8e4
F32 = mybir.dt.float32
EXP = mybir.ActivationFunctionType.Exp
DR = mybir.MatmulPerfMode.DoubleRow
WS = 64.0  # wq prescale: lifts w (std 0.02) out of e4m3 subnormal range
SCALE = 0.125 / (64.0 * 64.0)  # 1/sqrt(DV), undoing the q & k prescales
HILO = ((0, 0), (0, 1), (1, 0))  # (x, w) hi/lo product pairs

_CACHE = {}


def build_nc():
    nc = bacc.Bacc("TRN2", target_bir_lowering=False, debug=False)
    xq8_d = nc.dram_tensor("xq8", [2 * D, N], F8, kind="ExternalInput")
    xk8_d = nc.dram_tensor("xk8", [2 * D, N], F8, kind="ExternalInput")
    xv8_d = nc.dram_tensor("xv8", [2 * D, N], F8, kind="ExternalInput")
    wq8_d = nc.dram_tensor("wq8", [2 * D, DHC], F8, kind="ExternalInput")
    woT_d = nc.dram_tensor("woT", [DHC, D], BF, kind="ExternalInput")
    bq_d = nc.dram_tensor("bq", [DHC], F32, kind="ExternalInput")
    yT_d = nc.dram_tensor("yT", [D, N], mybir.dt.float16, kind="ExternalOutput")

    with tile.TileContext(nc) as tc:
        with (
            tc.tile_pool(name="consts", bufs=1) as consts,
            tc.tile_pool(name="xin", bufs=1) as xin,
            tc.tile_pool(name="prod", bufs=1) as prod,
            tc.tile_pool(name="work", bufs=3) as work,
            tc.tile_pool(name="norm", bufs=3) as norm,
            tc.tile_pool(name="yout", bufs=2) as yout,
            tc.tile_pool(name="ps", bufs=1, space="PSUM") as ps,
        ):
            # ---- weights + constants (small, load first) ----
            bq_pp = consts.tile([128, 2], F32, name="bq_pp")
            nc.sync.dma_start(
                out=bq_pp, in_=bq_d.ap().rearrange("(c p) -> p c", p=128)
            )
            bq_row = consts.tile([1, DHC], F32, name="bq_row")
            nc.sync.dma_start(
                out=bq_row, in_=bq_d.ap().rearrange("(a c) -> a c", a=1)
            )
            bq_bc = consts.tile([128, DHC], F32, name="bq_bc")
            nc.gpsimd.partition_broadcast(bq_bc, bq_row)
            utmask = consts.tile([128, 128], BF, name="utmask")
            make_upper_triangular(nc, utmask, val=1.0, diag=True)
            ident = consts.tile([128, 128], BF, name="ident")
            make_identity(nc, ident)

            # g axis = hl*DT + j: hi k-tiles then lo k-tiles
            wqT = consts.tile([128, 2 * DT, DHC], F8, name="wqT")
            nc.sync.dma_start(
                out=wqT, in_=wq8_d.ap().rearrange("(g p) c -> p g c", p=128)
            )

            # ---- bulk inputs: column-chunk DMAs ordered by first use ----
            xkT = xin.tile([128, 2 * DT, N], F8, name="xkT")
            xqT = xin.tile([128, 2 * DT, N], F8, name="xqT")
            xvT = xin.tile([128, 2 * DT, N], F8, name="xvT")

            def load_slice(t, d, n0, n1, gsplit=False):
                for g0, g1 in ((0, 8), (8, 16)) if gsplit else ((0, 16),):
                    nc.sync.dma_start(
                        out=t[:, g0:g1, n0:n1],
                        in_=d.ap()[g0 * 128 : g1 * 128, n0:n1].rearrange(
                            "(g p) n -> p g n", p=128
                        ),
                    )

            load_slice(xkT, xk8_d, 0, 512, gsplit=True)
            load_slice(xqT, xq8_d, 0, 512, gsplit=True)
            load_slice(xvT, xv8_d, 0, 512)
            load_slice(xkT, xk8_d, 512, 1024)
            load_slice(xkT, xk8_d, 1536, 2048)
            load_slice(xkT, xk8_d, 1024, 1536)
            load_slice(xqT, xq8_d, 1536, 2048)
            for s in range(1, 4):
                load_slice(xvT, xv8_d, s * 512, (s + 1) * 512)
            load_slice(xqT, xq8_d, 1024, 1536)
            load_slice(xqT, xq8_d, 512, 1024)
            woT = consts.tile([128, 2, D], BF, name="woT")
            nc.sync.dma_start(
                out=woT, in_=woT_d.ap().rearrange("(q p) c -> p q c", p=128)
            )

            # ---- persistent products ----
            vp = [
                prod.tile([128, HPC, DV + 1], BF, name=f"vp{m}")
                for m in range(NT)
            ]
            qT = [prod.tile([128, N], BF, name=f"qT{p}") for p in range(2)]
            kT = [prod.tile([128, N], BF, name=f"kT{p}") for p in range(2)]
            xaT = [prod.tile([128, N], BF, name=f"xaT{p}") for p in range(2)]

            def proj_qk(src_t, dst, c, p):
                pp = ps.tile([128, 512], F32, name="prj_qk", tag="prj", bufs=2)
                n_mm = len(HILO) * (DT // 2)
                i = 0
                for a, b in HILO:
                    for j2 in range(0, DT, 2):
                        nc.tensor.matmul(
                            pp,
                            wqT[:, b * DT + j2 : b * DT + j2 + 2,
                                p * 128 : (p + 1) * 128],
                            src_t[:, a * DT + j2 : a * DT + j2 + 2,
                                  c * 512 : (c + 1) * 512],
                            start=(i == 0),
                            stop=(i == n_mm - 1),
                            perf_mode=DR,
                        )
                        i += 1
                nc.vector.tensor_scalar_add(
                    dst[p][:, c * 512 : (c + 1) * 512], pp, bq_pp[:, p : p + 1]
                )

            def proj_v(m):
                pv = ps.tile([128, 512], F32, name="prj_v", tag="prj", bufs=2)
                pvv = pv[:, 0:DHC]
                n_mm = len(HILO) * (DT // 2)
                i = 0
                for a, b in HILO:
                    for j2 in range(0, DT, 2):
                        nc.tensor.matmul(
                            pvv,
                            xvT[:, a * DT + j2 : a * DT + j2 + 2,
                                m * 128 : (m + 1) * 128],
                            wqT[:, b * DT + j2 : b * DT + j2 + 2, :],
                            start=(i == 0),
                            stop=(i == n_mm - 1),
                            perf_mode=DR,
                        )
                        i += 1
                nc.vector.tensor_add(
                    vp[m][:, :, 0:DV],
                    pvv.rearrange("p (h d) -> p h d", h=HPC),
                    bq_bc.rearrange("p (h d) -> p h d", h=HPC),
                )
                nc.vector.memset(vp[m][:, :, DV : DV + 1], 1.0)

            def outproj_t(c, t, eng="v"):
                yp = ps.tile([128, 512], F32, name="yp", tag="prj", bufs=2)
                for p in range(2):
                    nc.tensor.matmul(
                        yp,
                        woT[:, p, t * 128 : (t + 1) * 128],
                        xaT[p][:, c * 512 : (c + 1) * 512],
                        start=(p == 0),
                        stop=(p == 1),
                    )
                y_sb = yout.tile(
                    [128, 512], mybir.dt.float16, name=f"y_sb{t}",
                    tag=f"y{t % 4}",
                )
                if eng == "a":
                    nc.scalar.copy(y_sb, yp)
                else:
                    nc.vector.tensor_copy(y_sb, yp)
                nc.sync.dma_start(
                    out=yT_d.ap()[
                        t * 128 : (t + 1) * 128, c * 512 : (c + 1) * 512
                    ],
                    in_=y_sb,
                )

            # pT tiles for unit u are consumed by the flipped PV next unit
            pT_tiles = {}
            xa_tiles = {}

            def s_exp_burst(c, hp, fillers):
                """S+exp burst for a HEAD PAIR (heads 2hp, 2hp+1).

                The two heads' S matmuls contract over disjoint PE row groups
                (array rows 0-63 vs 64-127, from the operands' base
                partitions). Both land in one [128,1024] psum and share a
                single wide exp."""
                jmax = 4 * c + 3
                fi = list(fillers)
                for j in range(jmax + 1):
                    off = max(0, (j - 4 * c) * 128)
                    w = 512 - off
                    sp = ps.tile([128, 1024], F32, name="sp", tag="sp", bufs=2)
                    pT = work.tile(
                        [128, 1024], BF, name="pT", tag="pT", bufs=22
                    )
                    for hr in range(2):
                        nc.tensor.matmul(
                            sp[:, hr * 512 : hr * 512 + w],
                            kT[hp][
                                hr * 64 : (hr + 1) * 64,
                                j * 128 : (j + 1) * 128,
                            ],
                            qT[hp][
                                hr * 64 : (hr + 1) * 64,
                                c * 512 + off : (c + 1) * 512,
                            ],
                            start=True,
                            stop=True,
                            skip_group_check=True,
                        )
                    if off:
                        # diag block: exp only the two valid [0,w) regions
                        spv = sp.rearrange("p (b k) -> p b k", b=2)[:, :, 0:w]
                        pTv = pT.rearrange("p (b k) -> p b k", b=2)[:, :, 0:w]
                        nc.scalar.activation(pTv, spv, EXP, scale=SCALE)
                    else:
                        nc.scalar.activation(pT, sp, EXP, scale=SCALE)
                    for hr in range(2):
                        if j >= 4 * c:  # diagonal block: causal mask
                            nc.vector.tensor_mul(
                                pT[:, hr * 512 : hr * 512 + 128],
                                pT[:, hr * 512 : hr * 512 + 128],
                                utmask,
                            )
                        pT_tiles[(c, 2 * hp + hr, j)] = pT[
                            :, hr * 512 : hr * 512 + 512
                        ]
                    if j % 2 and fi:
                        fi.pop(0)()
                for f in fi:
                    f()

            def pv_pair(c, hp):
                """Flipped PV for both heads of pair hp + normalization."""
                xa = norm.tile(
                    [128, 4, 128], BF, name="xa", tag="xa", bufs=3
                )
                xa_tiles[(c, hp)] = xa
                for hr in range(2):
                    h = 2 * hp + hr
                    op = ps.tile([128, 4, 128], F32, name="op", tag="op", bufs=2)
                    for qb in range(4):
                        jq = 4 * c + qb
                        for j in range(jq + 1):
                            off = max(0, (j - 4 * c) * 128)
                            col = qb * 128 - off
                            pT = pT_tiles[(c, h, j)]
                            nc.tensor.matmul(
                                op[:, qb, 0 : DV + 1],
                                pT[:, col : col + 128],
                                vp[j][:, h, :],
                                start=(j == 0),
                                stop=(j == jq),
                                skip_group_check=True,
                            )
                    rden = norm.tile(
                        [128, 4, 1], F32, name="rden", tag="rden", bufs=4
                    )
                    nc.vector.reciprocal(rden, op[:, :, DV : DV + 1])
                    nc.vector.tensor_mul(
                        xa[:, :, hr * 64 : (hr + 1) * 64],
                        op[:, :, 0:DV],
                        rden.broadcast_to([128, 4, DV]),
                    )
                for hr in range(2):
                    for j in range(4 * c + 4):
                        del pT_tiles[(c, 2 * hp + hr, j)]

            def transp_pair(c, hp):
                xa = xa_tiles.pop((c, hp))
                tp = ps.tile([128, 4, 128], BF, name="tp", tag="op", bufs=2)
                for qb in range(4):
                    nc.tensor.matmul(
                        tp[:, qb, :],
                        xa[:, qb, :],
                        ident,
                        is_transpose=True,
                    )
                nc.vector.tensor_copy(xaT[hp][:, c * 512 : (c + 1) * 512], tp)

            def F(fn, *a):
                return lambda: fn(*a)

            fillers = {
                (0, 0): [
                    F(proj_qk, xkT, kT, 1, 0),
                    F(proj_qk, xkT, kT, 1, 1),
                    F(proj_qk, xkT, kT, 3, 0),
                    F(proj_qk, xkT, kT, 3, 1),
                ],
                (0, 1): [
                    F(proj_qk, xkT, kT, 2, 0),
                    F(proj_qk, xkT, kT, 2, 1),
                    F(proj_qk, xqT, qT, 3, 0),
                    F(proj_qk, xqT, qT, 3, 1),
                ],
                (3, 0): [F(proj_v, m) for m in range(4, 16)],
                (3, 1): [F(proj_qk, xqT, qT, 2, 0), F(proj_qk, xqT, qT, 2, 1)],
                (2, 0): [F(outproj_t, 0, t, "va"[t % 2]) for t in range(4)],
                (2, 1): [F(outproj_t, 0, t, "va"[t % 2]) for t in range(4, 8)]
                + [F(proj_qk, xqT, qT, 1, 0), F(proj_qk, xqT, qT, 1, 1)],
                (1, 0): [F(outproj_t, 3, t, "va"[t % 2]) for t in range(8)],
                (1, 1): [F(outproj_t, 2, t, "va"[t % 2]) for t in range(8)],
            }

            # prologue: chunk-0 projections; later kT/qT chunks are fillers
            for p in range(2):
                proj_qk(xkT, kT, 0, p)
            for p in range(2):
                proj_qk(xqT, qT, 0, p)
            s_exp_burst(0, 0, fillers[(0, 0)])
            for m in range(4):
                proj_v(m)

            units = [(0, 1), (3, 0), (3, 1), (2, 0), (2, 1), (1, 0), (1, 1)]
            prev = (0, 0)
            for cu in units:
                s_exp_burst(*cu, fillers[cu])
                pv_pair(*prev)
                transp_pair(*prev)
                prev = cu
            pv_pair(*prev)
            transp_pair(*prev)
            for i, t in enumerate(range(DT)):
                outproj_t(1, t, "av"[i % 2])
    nc.compile()
    return nc


def kernel(**inputs):
    inputs = {k: np.asarray(v) for k, v in inputs.items()}
    Q, K, V = inputs["Q"], inputs["K"], inputs["V"]
    wq, bq, wo, bo = inputs["wq"], inputs["bq"], inputs["wo"], inputs["bo"]

    F8NP = ml_dtypes.float8_e4m3

    def bfT(x):  # bf16 transpose [n, d] -> [d, n]
        return np.ascontiguousarray(x.astype(ml_dtypes.bfloat16).T)

    def hilo8(x):  # [n, d] f32 -> [2d, n] fp8: hi rows then lo rows
        xT = np.ascontiguousarray(x.T, dtype=np.float32)
        hi = xT.astype(F8NP)
        lo = (xT - hi.astype(np.float32)).astype(F8NP)
        return np.ascontiguousarray(np.concatenate([hi, lo], axis=0))

    xq8 = [hilo8(Q[b]) for b in range(B)]
    xk8 = [hilo8(K[b]) for b in range(B)]
    xv8 = [hilo8(V[b]) for b in range(B)]
    # wq prescaled by WS for fp8; v picks up WS, undone in wo; q.k picks up
    # WS^2, undone in the exp scale
    wq8 = [hilo8(wq[g * DHC : (g + 1) * DHC, :] * WS) for g in range(4)]
    woT = [bfT(wo[:, g * DHC : (g + 1) * DHC] * (1.0 / WS)) for g in range(4)]
    bqs = [np.ascontiguousarray(bq[g * DHC : (g + 1) * DHC] * WS,
                                dtype=np.float32)
           for g in range(4)]

    if "nc" not in _CACHE:
        _CACHE["nc"] = build_nc()
    nc = _CACHE["nc"]

    in_maps = []
    for core in range(8):
        b, g = divmod(core, 4)
        in_maps.append(
            {
                "xq8": xq8[b],
                "xk8": xk8[b],
                "xv8": xv8[b],
                "wq8": wq8[g],
                "woT": woT[g],
                "bq": bqs[g],
            }
        )
    import os

    trace = bool(int(os.environ.get("KERNEL_TRACE", "0")))
    try:
        res = run_bass_kernel_spmd(
            nc, in_maps, core_ids=list(range(8)), trace=trace
        )
    except ModuleNotFoundError:
        # NTFF profiling hook unavailable in this environment
        res = run_bass_kernel_spmd(nc, in_maps, core_ids=list(range(8)))
    _CACHE["last_results"] = res

    out = np.empty((B, N, D), np.float32)
    for b in range(B):
        acc = res.results[4 * b]["yT"].astype(np.float32)
        for g in range(1, 4):
            acc += res.results[4 * b + g]["yT"]
        out[b] = acc.T + bo
    return out


# revision 38
# speedup vs baseline: 1.2318x; 1.0562x over previous
"""Multi-head attention (B=2, N=2048, d_model=1024, H=16) on 8 NeuronCores.

Sharding: data-parallel on batch (2) x tensor-parallel on heads (4 groups of
4 heads). Core c handles batch c//4, head-group c%4. Each core computes its
heads' Q/K/V projections, causal attention, and a partial output projection;
the host sums the 4 partials per batch.

Key structure (v2):
- Q/K/V projections run in fp8e4 DoubleRow perf mode with hi/lo error
  compensation: x ~ x_hi + x_lo, w ~ w_hi + w_lo (all e4m3, split on host),
  and the three products hh+hl+lh accumulate in one PSUM group. 2 k-tiles
  pack per DoubleRow matmul at 0.5 cycles/row -> ~2.7x faster than bf16 at
  bf16-like accuracy (lo*lo term ~0.1%).
- Scores are computed in S.T orientation [keys, queries] per head-pair
  (disjoint PE row groups), exp'd on ACT without max-subtraction (scores
  bounded), causal-masked by an upper-tri multiply on the diagonal blocks.
- PV is "flipped": P blocks [128k x 128q] are the stationary operand and
  the (bf16) V tile [128k x 65] streams, so each key-block step costs 65
  rows instead of 512. Column 64 of V is ones -> softmax denominator rides
  along. Output lands as [q, dh] + per-partition scalar normalize on DVE,
  then a PE transpose puts it back in [dh, q] for the output projection.
- Output projection in bf16; PSUM->SBUF f16 copies split across DVE /
  GPSIMD / ACT; partial sums of the 4 head-groups folded on host.
"""

import sys

if "/opt/trn_rl_repo" not in sys.path:
    sys.path.insert(0, "/opt/trn_rl_repo")

import numpy as np
import ml_dtypes

import concourse.bass as bass
import concourse.mybir as mybir
import concourse.tile as tile
from concourse import bacc
from concourse.bass_utils import run_bass_kernel_spmd
from concourse.masks import make_upper_triangular, make_identity

B, N, D, H = 2, 2048, 1024, 16
DV = D // H  # 64
HPC = H // 4  # heads per core: 4
DHC = HPC * DV  # head dims per core: 256
NT = N // 128  # 16 m-tiles
DT = D // 128  # 8 din-tiles
BF = mybir.dt.bfloat16
F8 = mybir.dt.float8e4
F32 = mybir.dt.float32
EXP = mybir.ActivationFunctionType.Exp
DR = mybir.MatmulPerfMode.DoubleRow
WS = 32.0  # wq prescale: above e4m3 subnormals, below e4m3 max for q/k
SCALE = 0.125 / (32.0 * 32.0)  # 1/sqrt(DV), undoing the q & k prescales
HILO = ((0, 0), (0, 1), (1, 0))  # (x, w) hi/lo product pairs

_CACHE = {}


def build_nc():
    nc = bacc.Bacc("TRN2", target_bir_lowering=False, debug=False)
    xq8_d = nc.dram_tensor("xq8", [2 * D, N], F8, kind="ExternalInput")
    xk8_d = nc.dram_tensor("xk8", [2 * D, N], F8, kind="ExternalInput")
    xv8_d = nc.dram_tensor("xv8", [2 * D, N], F8, kind="ExternalInput")
    wq8_d = nc.dram_tensor("wq8", [2 * D, DHC], F8, kind="ExternalInput")
    woT_d = nc.dram_tensor("woT", [DHC, D], BF, kind="ExternalInput")
    bq_d = nc.dram_tensor("bq", [DHC], F32, kind="ExternalInput")
    yT_d = nc.dram_tensor("yT", [D, N], mybir.dt.float16, kind="ExternalOutput")

    with tile.TileContext(nc) as tc:
        with (
            tc.tile_pool(name="consts", bufs=1) as consts,
            tc.tile_pool(name="xin", bufs=1) as xin,
            tc.tile_pool(name="prod", bufs=1) as prod,
            tc.tile_pool(name="work", bufs=3) as work,
            tc.tile_pool(name="norm", bufs=3) as norm,
            tc.tile_pool(name="yout", bufs=2) as yout,
            tc.tile_pool(name="ps", bufs=1, space="PSUM") as ps,
        ):
            # ---- weights + constants (small, load first) ----
            bq_pp = consts.tile([128, 2], F32, name="bq_pp")
            nc.sync.dma_start(
                out=bq_pp, in_=bq_d.ap().rearrange("(c p) -> p c", p=128)
            )
            bq_row = consts.tile([1, DHC], F32, name="bq_row")
            nc.sync.dma_start(
                out=bq_row, in_=bq_d.ap().rearrange("(a c) -> a c", a=1)
            )
            bq_bc = consts.tile([128, DHC], F32, name="bq_bc")
            nc.gpsimd.partition_broadcast(bq_bc, bq_row)
            utmask = consts.tile([128, 128], BF, name="utmask")
            make_upper_triangular(nc, utmask, val=1.0, diag=True)
            ident = consts.tile([128, 128], BF, name="ident")
            make_identity(nc, ident)

            # g axis = hl*DT + j: hi k-tiles then lo k-tiles
            wqT = consts.tile([128, 2 * DT, DHC], F8, name="wqT")
            for g0, g1 in ((0, 8), (8, 16)):
                nc.sync.dma_start(
                    out=wqT[:, g0:g1, :],
                    in_=wq8_d.ap()[g0 * 128 : g1 * 128, :].rearrange(
                        "(g p) c -> p g c", p=128
                    ),
                )

            # ---- bulk inputs: column-chunk DMAs ordered by first use ----
            xkT = xin.tile([128, 2 * DT, N], F8, name="xkT")
            xqT = xin.tile([128, 2 * DT, N], F8, name="xqT")
            xvT = xin.tile([128, 2 * DT, N], F8, name="xvT")

            def load_slice(t, d, n0, n1, gsplit=False):
                for g0, g1 in ((0, 8), (8, 16)) if gsplit else ((0, 16),):
                    nc.sync.dma_start(
                        out=t[:, g0:g1, n0:n1],
                        in_=d.ap()[g0 * 128 : g1 * 128, n0:n1].rearrange(
                            "(g p) n -> p g n", p=128
                        ),
                    )

            load_slice(xkT, xk8_d, 0, 512, gsplit=True)
            load_slice(xqT, xq8_d, 0, 512, gsplit=True)
            load_slice(xqT, xq8_d, 1536, 2048, gsplit=True)
            load_slice(xkT, xk8_d, 512, 1024)
            load_slice(xkT, xk8_d, 1536, 2048)
            load_slice(xvT, xv8_d, 0, 512)
            load_slice(xkT, xk8_d, 1024, 1536)
            for s in range(1, 4):
                load_slice(xvT, xv8_d, s * 512, (s + 1) * 512)
            load_slice(xqT, xq8_d, 1024, 1536)
            load_slice(xqT, xq8_d, 512, 1024)
            woT = consts.tile([128, 2, D], BF, name="woT")
            nc.sync.dma_start(
                out=woT, in_=woT_d.ap().rearrange("(q p) c -> p q c", p=128)
            )

            # ---- persistent products ----
            vp = [
                prod.tile([128, HPC, DV + 1], BF, name=f"vp{m}")
                for m in range(NT)
            ]
            qT = [prod.tile([128, N], F8, name=f"qT{p}") for p in range(2)]
            kT = [
                prod.tile([128, 2, N], F8, name=f"kT{p}") for p in range(2)
            ]
            xaT = [prod.tile([128, N], BF, name=f"xaT{p}") for p in range(2)]

            def proj_qk(src_t, dst, c, p, hilo=False):
                pp = ps.tile([128, 512], F32, name="prj_qk", tag="prj", bufs=2)
                n_mm = len(HILO) * (DT // 2)
                i = 0
                for a, b in HILO:
                    for j2 in range(0, DT, 2):
                        nc.tensor.matmul(
                            pp,
                            wqT[:, b * DT + j2 : b * DT + j2 + 2,
                                p * 128 : (p + 1) * 128],
                            src_t[:, a * DT + j2 : a * DT + j2 + 2,
                                  c * 512 : (c + 1) * 512],
                            start=(i == 0),
                            stop=(i == n_mm - 1),
                            perf_mode=DR,
                        )
                        i += 1
                if hilo:
                    hi = dst[p][:, 0, c * 512 : (c + 1) * 512]
                    nc.vector.tensor_scalar_add(hi, pp, bq_pp[:, p : p + 1])
                    nc.vector.tensor_sub(
                        dst[p][:, 1, c * 512 : (c + 1) * 512], pp, hi
                    )
                else:
                    nc.vector.tensor_scalar_add(
                        dst[p][:, c * 512 : (c + 1) * 512],
                        pp,
                        bq_pp[:, p : p + 1],
                    )

            def proj_v(m):
                pv = ps.tile([128, 512], F32, name="prj_v", tag="prj", bufs=2)
                pvv = pv[:, 0:DHC]
                n_mm = len(HILO) * (DT // 2)
                i = 0
                for a, b in HILO:
                    for j2 in range(0, DT, 2):
                        nc.tensor.matmul(
                            pvv,
                            xvT[:, a * DT + j2 : a * DT + j2 + 2,
                                m * 128 : (m + 1) * 128],
                            wqT[:, b * DT + j2 : b * DT + j2 + 2, :],
                            start=(i == 0),
                            stop=(i == n_mm - 1),
                            perf_mode=DR,
                        )
                        i += 1
                nc.vector.tensor_add(
                    vp[m][:, :, 0:DV],
                    pvv.rearrange("p (h d) -> p h d", h=HPC),
                    bq_bc.rearrange("p (h d) -> p h d", h=HPC),
                )
                nc.vector.memset(vp[m][:, :, DV : DV + 1], 1.0)

            def outproj_t(c, t, eng="v", tag="prj"):
                yp = ps.tile([128, 512], F32, name="yp", tag=tag, bufs=2)
                for p in range(2):
                    nc.tensor.matmul(
                        yp,
                        woT[:, p, t * 128 : (t + 1) * 128],
                        xaT[p][:, c * 512 : (c + 1) * 512],
                        start=(p == 0),
                        stop=(p == 1),
                    )
                y_sb = yout.tile(
                    [128, 512], mybir.dt.float16, name=f"y_sb{t}",
                    tag=f"y{t % 4}",
                )
                if eng == "a":
                    nc.scalar.copy(y_sb, yp)
                elif eng == "va":  # latency-critical: halves on both engines
                    nc.vector.tensor_copy(y_sb[:, 0:256], yp[:, 0:256])
                    nc.scalar.copy(y_sb[:, 256:512], yp[:, 256:512])
                else:
                    nc.vector.tensor_copy(y_sb, yp)
                nc.sync.dma_start(
                    out=yT_d.ap()[
                        t * 128 : (t + 1) * 128, c * 512 : (c + 1) * 512
                    ],
                    in_=y_sb,
                )

            # pT tiles for unit u are consumed by the flipped PV next unit
            pT_tiles = {}
            xa_tiles = {}

            def s_exp_burst(c, hp, fillers):
                """S+exp burst for a HEAD PAIR (heads 2hp, 2hp+1).

                The two heads' S matmuls contract over disjoint PE row groups
                (array rows 0-63 vs 64-127, from the operands' base
                partitions). Both land in one [128,1024] psum and share a
                single wide exp."""
                jmax = 4 * c + 3
                fi = list(fillers)
                for j in range(jmax + 1):
                    off = max(0, (j - 4 * c) * 128)
                    w = 512 - off
                    sp = ps.tile([128, 1024], F32, name="sp", tag="sp", bufs=2)
                    pT = work.tile(
                        [128, 1024], BF, name="pT", tag="pT", bufs=24
                    )
                    for hr in range(2):
                        qmv = (
                            qT[hp][
                                hr * 64 : (hr + 1) * 64,
                                c * 512 + off : (c + 1) * 512,
                            ]
                            .unsqueeze(1)
                            .broadcast_to([64, 2, w])
                        )
                        nc.tensor.matmul(
                            sp[:, hr * 512 : hr * 512 + w],
                            kT[hp][
                                hr * 64 : (hr + 1) * 64,
                                :,
                                j * 128 : (j + 1) * 128,
                            ],
                            qmv,
                            start=True,
                            stop=True,
                            perf_mode=DR,
                            skip_group_check=True,
                        )
                    if off:
                        # diag block: exp only the two valid [0,w) regions
                        spv = sp.rearrange("p (b k) -> p b k", b=2)[:, :, 0:w]
                        pTv = pT.rearrange("p (b k) -> p b k", b=2)[:, :, 0:w]
                        nc.scalar.activation(pTv, spv, EXP, scale=SCALE)
                    else:
                        nc.scalar.activation(pT, sp, EXP, scale=SCALE)
                    for hr in range(2):
                        if j >= 4 * c:  # diagonal block: causal mask
                            nc.vector.tensor_mul(
                                pT[:, hr * 512 : hr * 512 + 128],
                                pT[:, hr * 512 : hr * 512 + 128],
                                utmask,
                            )
                        pT_tiles[(c, 2 * hp + hr, j)] = pT[
                            :, hr * 512 : hr * 512 + 512
                        ]
                    if j % 2 and fi:
                        fi.pop(0)()
                for f in fi:
                    f()

            def pv_mms(c, h, qb, op):
                jq = 4 * c + qb
                for j in range(jq + 1):
                    off = max(0, (j - 4 * c) * 128)
                    col = qb * 128 - off
                    pT = pT_tiles[(c, h, j)]
                    nc.tensor.matmul(
                        op[:, qb, 0 : DV + 1],
                        pT[:, col : col + 128],
                        vp[j][:, h, :],
                        start=(j == 0),
                        stop=(j == jq),
                        skip_group_check=True,
                    )

            def pv_pair(c, hp):
                """Flipped PV for both heads of pair hp + normalization."""
                xa = norm.tile(
                    [128, 4, 128], BF, name="xa", tag="xa", bufs=3
                )
                xa_tiles[(c, hp)] = xa
                for hr in range(2):
                    h = 2 * hp + hr
                    op = ps.tile([128, 4, 128], F32, name="op", tag="op", bufs=2)
                    for qb in range(4):
                        pv_mms(c, h, qb, op)
                    rden = norm.tile(
                        [128, 4, 1], F32, name="rden", tag="rden", bufs=4
                    )
                    nc.vector.reciprocal(rden, op[:, :, DV : DV + 1])
                    nc.vector.tensor_mul(
                        xa[:, :, hr * 64 : (hr + 1) * 64],
                        op[:, :, 0:DV],
                        rden.broadcast_to([128, 4, DV]),
                    )
                for hr in range(2):
                    for j in range(4 * c + 4):
                        del pT_tiles[(c, 2 * hp + hr, j)]

            def pv_pair_fine(c, hp):
                """Per-qb PV+norm: the last pair's xa fills per query block
                as its exps land, so only the transpose trails the last exp."""
                xa = norm.tile(
                    [128, 4, 128], BF, name="xa", tag="xa", bufs=3
                )
                xa_tiles[(c, hp)] = xa
                ops = [
                    ps.tile([128, 4, 128], F32, name="op", tag="op", bufs=2)
                    for _ in range(2)
                ]
                rdens = [
                    norm.tile([128, 4, 1], F32, name="rden", tag="rden", bufs=4)
                    for _ in range(2)
                ]
                for qb in range(4):
                    for hr in range(2):
                        pv_mms(c, 2 * hp + hr, qb, ops[hr])
                    for hr in range(2):
                        nc.vector.reciprocal(
                            rdens[hr][:, qb, :], ops[hr][:, qb, DV : DV + 1]
                        )
                        nc.vector.tensor_mul(
                            xa[:, qb, hr * 64 : (hr + 1) * 64],
                            ops[hr][:, qb, 0:DV],
                            rdens[hr][:, qb, :].broadcast_to([128, DV]),
                        )
                for hr in range(2):
                    for j in range(4 * c + 4):
                        del pT_tiles[(c, 2 * hp + hr, j)]

            def transp_pair(c, hp):
                xa = xa_tiles.pop((c, hp))
                tp = ps.tile([128, 4, 128], BF, name="tp", tag="op", bufs=2)
                for qb in range(4):
                    nc.tensor.matmul(
                        tp[:, qb, :],
                        xa[:, qb, :],
                        ident,
                        is_transpose=True,
                    )
                nc.vector.tensor_copy(xaT[hp][:, c * 512 : (c + 1) * 512], tp)

            def F(fn, *a):
                return lambda: fn(*a)

            fillers = {
                (0, 0): [
                    F(proj_qk, xkT, kT, 1, 0, True),
                    F(proj_qk, xkT, kT, 1, 1, True),
                    F(proj_qk, xkT, kT, 3, 0, True),
                    F(proj_qk, xkT, kT, 3, 1, True),
                ],
                (0, 1): [
                    F(proj_qk, xkT, kT, 2, 0, True),
                    F(proj_qk, xkT, kT, 2, 1, True),
                    F(proj_qk, xqT, qT, 3, 0),
                    F(proj_qk, xqT, qT, 3, 1),
                ],
                (3, 0): [F(proj_v, m) for m in range(4, 16)],
                (3, 1): [
                    F(proj_qk, xqT, qT, 2, 0),
                    F(proj_qk, xqT, qT, 2, 1),
                ],
                (2, 0): [F(outproj_t, 0, t) for t in range(4)]
                + [F(proj_qk, xqT, qT, 1, 0), F(proj_qk, xqT, qT, 1, 1)],
                (2, 1): [F(outproj_t, 0, t) for t in range(4, 8)]
                + [F(outproj_t, 3, t) for t in range(4)],
                (1, 0): [F(outproj_t, 3, t, "va") for t in range(4, 8)],
                (1, 1): [F(outproj_t, 2, t, "va") for t in range(8)],
            }

            # prologue: chunk-0 projections; later kT/qT chunks are fillers
            for p in range(2):
                proj_qk(xkT, kT, 0, p, hilo=True)
            for p in range(2):
                proj_qk(xqT, qT, 0, p)
            s_exp_burst(0, 0, fillers[(0, 0)])
            for m in range(4):
                proj_v(m)

            units = [(0, 1), (3, 0), (3, 1), (2, 0), (2, 1), (1, 0), (1, 1)]
            prev = (0, 0)
            for cu in units:
                s_exp_burst(*cu, fillers[cu])
                pv_pair(*prev)
                transp_pair(*prev)
                prev = cu
            pv_pair_fine(*prev)
            transp_pair(*prev)
            # tail chunk: batch y output into 2 quad-DMAs (SP-SEQ issue is
            # the post-PE critical path at ~700ns per DMA)
            for q in range(2):
                yq = yout.tile(
                    [128, 4, 512], mybir.dt.float16, name=f"yq{q}",
                    tag=f"yq{q}", bufs=1,
                )
                for u in range(4):
                    t = 4 * q + u
                    yp = ps.tile(
                        [128, 512], F32, name="yp",
                        tag="sp" if t % 2 else "prj", bufs=2,
                    )
                    for p in range(2):
                        nc.tensor.matmul(
                            yp,
                            woT[:, p, t * 128 : (t + 1) * 128],
                            xaT[p][:, 512:1024],
                            start=(p == 0),
                            stop=(p == 1),
                        )
                    nc.vector.tensor_copy(yq[:, u, 0:256], yp[:, 0:256])
                    nc.scalar.copy(yq[:, u, 256:512], yp[:, 256:512])
                eng = nc.sync if q == 0 else nc.vector
                eng.dma_start(
                    out=yT_d.ap()[
                        q * 512 : (q + 1) * 512, 512:1024
                    ].rearrange("(u p) n -> p u n", p=128),
                    in_=yq,
                )

    nc.compile()
    return nc


def kernel(**inputs):
    inputs = {k: np.asarray(v) for k, v in inputs.items()}
    Q, K, V = inputs["Q"], inputs["K"], inputs["V"]
    wq, bq, wo, bo = inputs["wq"], inputs["bq"], inputs["wo"], inputs["bo"]

    F8NP = ml_dtypes.float8_e4m3

    def bfT(x):  # bf16 transpose [n, d] -> [d, n]
        return np.ascontiguousarray(x.astype(ml_dtypes.bfloat16).T)

    def hilo8(x):  # [n, d] f32 -> [2d, n] fp8: hi rows then lo rows
        xT = np.ascontiguousarray(x.T, dtype=np.float32)
        hi = xT.astype(F8NP)
        lo = (xT - hi.astype(np.float32)).astype(F8NP)
        return np.ascontiguousarray(np.concatenate([hi, lo], axis=0))

    xq8 = [hilo8(Q[b]) for b in range(B)]
    xk8 = [hilo8(K[b]) for b in range(B)]
    xv8 = [hilo8(V[b]) for b in range(B)]
    # wq prescaled by WS for fp8; v picks up WS, undone in wo; q.k picks up
    # WS^2, undone in the exp scale
    wq8 = [hilo8(wq[g * DHC : (g + 1) * DHC, :] * WS) for g in range(4)]
    woT = [bfT(wo[:, g * DHC : (g + 1) * DHC] * (1.0 / WS)) for g in range(4)]
    bqs = [np.ascontiguousarray(bq[g * DHC : (g + 1) * DHC] * WS,
                                dtype=np.float32)
           for g in range(4)]

    if "nc" not in _CACHE:
        _CACHE["nc"] = build_nc()
    nc = _CACHE["nc"]

    in_maps = []
    for core in range(8):
        b, g = divmod(core, 4)
        in_maps.append(
            {
                "xq8": xq8[b],
                "xk8": xk8[b],
                "xv8": xv8[b],
                "wq8": wq8[g],
                "woT": woT[g],
                "bq": bqs[g],
            }
        )
    import os

    trace = bool(int(os.environ.get("KERNEL_TRACE", "0")))
    try:
        res = run_bass_kernel_spmd(
            nc, in_maps, core_ids=list(range(8)), trace=trace
        )
    except ModuleNotFoundError:
        # NTFF profiling hook unavailable in this environment
        res = run_bass_kernel_spmd(nc, in_maps, core_ids=list(range(8)))
    _CACHE["last_results"] = res

    out = np.empty((B, N, D), np.float32)
    for b in range(B):
        acc = res.results[4 * b]["yT"].astype(np.float32)
        for g in range(1, 4):
            acc += res.results[4 * b + g]["yT"]
        out[b] = acc.T + bo
    return out
